# revision 49
# baseline (speedup 1.0000x reference)
"""Trainium2 Bass kernel for AttentionWithGeGLU pooling.

Math (per batch row b):
  q[s]   = sum_d x[b,s,d]^2
  rs[s]  = (q/D + eps)^-1/2
  t[s]   = sum_d x[b,s,d] * (ln_w*att_w)[d]
  score  = rs * t            (att_b dropped: softmax is shift-invariant)
  e      = exp(score);  denom = sum_s e
  pooled[b,d] = ln_w[d]/denom * sum_s (e[s]*rs[s]) * x[b,s,d]
  h      = pooled @ geglu_w + geglu_b;  out = val * gelu(gate)

Sharding: two NEFF launches (collectives are unavailable on this runtime):
  A) data-parallel pooling over batch (4 batches/core), transposed-layout
     kernel (_build_nc_pool_t): host ships x as bf16 [NB, D, S]; q/t are
     computed on the PE via "flip" matmuls (lhsT = strided 128x128 x-block,
     rhs = a-chunk / ones, out = one [128,1] psum column per s-block) so
     the DVE never row-reduces for the scores; squares on ACT (+1 on Pool);
     softmax smalls on [128,16] tiles; the softmax weight row c is
     materialized without any transpose engine support via a contiguous
     DRAM write+read roundtrip (the strided lhsT slicing makes c's natural
     flat order equal x's free-dim order) + Pool partition_broadcast;
     pooled chunks = Pool tensor_mul + DVE reduce per 128-d chunk.
     Work is software-pipelined over 8 (batch, s-half) units.
  B) tensor-parallel GeGLU: host gathers+transposes pooled (128 KB), each
     core computes its 512 matching val+gate columns with bf16 weights.

Engine budget per core (cost model): DVE 82us (pooled reduces — the wall),
Pool 70us, ACT 53us, DMA ~50us, PE ~free (flips are 1-row outputs).
NOTE this runtime NRT-faults on custom-DVE ops (tensor_tensor_reduce), PE
transpose, and fp8 matmuls — all avoided here; see memory notes.
"""

import os
import numpy as np

B, S, D, OUT = 32, 2048, 1024, 4096
EPS = 1e-6
NCORES = 8
NB = B // NCORES          # batches per core
COLS = OUT // NCORES      # val columns per core
P = 128
NT = S // P               # seq tiles per batch

_cache = {}

# sim_time.py sets this: CoreSim's executor lacks Gelu numerics, so sim
# builds substitute Tanh (timing-identical table-based activation).
SIM_SAFE_GELU = False


def _build_nc_pool(mm="xbf16", dve_q_every=8):
    """Pooling NEFF. mm="xbf16": x arrives host-converted to bf16 (halves
    HBM traffic); q/t/pooled computed from bf16 x with fp32 accumulation.
    Every `dve_q_every`-th tile computes q on DVE instead of ACT to balance
    the two engines."""
    import concourse.bacc as bacc
    import concourse.mybir as mybir
    import concourse.tile as tile
    from contextlib import ExitStack

    f32 = mybir.dt.float32
    bf16 = mybir.dt.bfloat16
    xdt = bf16 if mm == "xbf16" else f32
    AF = mybir.ActivationFunctionType
    OP = mybir.AluOpType
    AX = mybir.AxisListType

    nc = bacc.Bacc(
        "TRN2",
        target_bir_lowering=False,
        debug=False,
        enable_asserts=False,
        num_devices=NCORES,
    )

    GRP = 4          # tiles per softmax/matmul group; one DMA per group
    NG = NT // GRP   # groups per batch

    x_d = nc.dram_tensor("x", [NB, S, D], xdt, kind="ExternalInput").ap()
    a_d = nc.dram_tensor("a", [1, D], xdt, kind="ExternalInput").ap()
    lnw_d = nc.dram_tensor("lnw", [1, D], f32, kind="ExternalInput").ap()
    cst_d = nc.dram_tensor("cst", [1, 2], f32, kind="ExternalInput").ap()
    pooled_d = nc.dram_tensor("pooled", [NB, D], f32, kind="ExternalOutput").ap()

    with tile.TileContext(nc) as tc, ExitStack() as ctx:
        singles = ctx.enter_context(tc.tile_pool(name="singles", bufs=1))
        xpool = ctx.enter_context(tc.tile_pool(name="xp", bufs=7))
        scratch = ctx.enter_context(tc.tile_pool(name="scr", bufs=2))
        small = ctx.enter_context(tc.tile_pool(name="small", bufs=3))
        psum_pool = ctx.enter_context(
            tc.tile_pool(name="pspool", bufs=2, space="PSUM")
        )
        psum_small = ctx.enter_context(
            tc.tile_pool(name="pssm", bufs=2, space="PSUM")
        )

        if os.environ.get("KERNEL_TABLELOAD", "0") == "1":
            # Preload the one act-table set containing Square+Ln+Exp so the
            # table-load fixpoint doesn't thrash between per-func sets.
            from concourse.hw_specs import get_activation_tables
            _tables = get_activation_tables(nc.m.arch)
            _set_id = list(_tables).index("natural_log_exp_and_others")
            _ld = mybir.InstLoadActFuncSet(
                name=nc.get_next_instruction_name(), ins=[], outs=[],
                act_func_set_id=_set_id,
            )
            nc.scalar.add_instruction(_ld)

        a_bc = singles.tile([P, D], xdt)
        nc.sync.dma_start(out=a_bc, in_=a_d.to_broadcast([P, D]))
        lnw_sb = singles.tile([1, D], f32)
        nc.sync.dma_start(out=lnw_sb, in_=lnw_d)
        # constants via DMA broadcast (DVE memset is unreliable on this runtime)
        ones = singles.tile([P, 1], f32)
        nc.sync.dma_start(out=ones, in_=cst_d[0:1, 0:1].to_broadcast([P, 1]))
        eps_col = singles.tile([P, 1], f32)
        nc.sync.dma_start(out=eps_col, in_=cst_d[0:1, 1:2].to_broadcast([P, 1]))

        pooled_sb = singles.tile([1, NB, D], f32)

        for b in range(NB):
            q_all = small.tile([P, NT], f32, tag="q")
            t_all = small.tile([P, NT], f32, tag="t")
            e_all = small.tile([P, NT], f32, tag="e")
            pp = psum_pool.tile([1, D], f32, tag="acc")
            for g in range(NG):
                xt = xpool.tile([P, GRP, D], xdt, tag="x")
                if os.environ.get("KERNEL_GRPDMA", "0") == "1":
                    nc.sync.dma_start(
                        out=xt,
                        in_=x_d[b, g * GRP * P:(g + 1) * GRP * P, :].rearrange(
                            "(grp p) d -> p grp d", p=P
                        ),
                    )
                else:
                    for jj in range(GRP):
                        j = g * GRP + jj
                        nc.sync.dma_start(
                            out=xt[:, jj, :],
                            in_=x_d[b, j * P:(j + 1) * P, :],
                        )
                for jj in range(GRP):
                    j = g * GRP + jj
                    # q: ACT square (plain), then DVE row-reduce.
                    # The accum_out fast path is NRT-fatal on this runtime.
                    sq = scratch.tile([P, D], xdt, tag="sq")
                    nc.scalar.activation(out=sq, in_=xt[:, jj, :],
                                         func=AF.Square)
                    nc.vector.reduce_sum(q_all[:, j:j + 1], sq, axis=AX.X)
                    tp = scratch.tile([P, D], xdt, tag="tp")
                    nc.vector.tensor_mul(tp, xt[:, jj, :], a_bc)
                    nc.vector.reduce_sum(t_all[:, j:j + 1], tp, axis=AX.X)

                gs = slice(g * GRP, (g + 1) * GRP)
                # rs = (q/D + eps)^-1/2 via fast-inverse-sqrt + 3 Newton
                # steps on DVE (avoids Ln/Exp table traffic; Exp for the
                # softmax is then the only other ACT function in use and
                # shares Square's table set).
                v = small.tile([P, GRP], f32, tag="v")
                nc.vector.tensor_scalar(
                    out=v, in0=q_all[:, gs], scalar1=1.0 / D, scalar2=EPS,
                    op0=OP.mult, op1=OP.add)
                # v = mean(x^2)+eps is ~1 for unit-variance rows, so Newton
                # from the first iterate y1 = 1.5 - 0.5*v converges fast.
                y = small.tile([P, GRP], f32, tag="y")
                nc.vector.tensor_scalar(
                    out=y, in0=v, scalar1=-0.5, scalar2=1.5,
                    op0=OP.mult, op1=OP.add)
                for _ in range(3):
                    u = small.tile([P, GRP], f32, tag="u")
                    nc.vector.tensor_mul(u, y, y)
                    nc.vector.tensor_mul(u, u, v)
                    nc.vector.tensor_scalar(
                        out=u, in0=u, scalar1=-0.5, scalar2=1.5,
                        op0=OP.mult, op1=OP.add)
                    nc.vector.tensor_mul(y, y, u)
                rs = y
                sc = small.tile([P, GRP], f32, tag="sc")
                nc.vector.tensor_mul(sc, t_all[:, gs], rs)
                nc.scalar.activation(out=e_all[:, gs], in_=sc, func=AF.Exp)
                c_g = small.tile([P, GRP], xdt, tag="c")
                nc.vector.tensor_mul(c_g, e_all[:, gs], rs)

                # pass B for this group: pooled_raw[1, D] += c_j^T @ x_j
                for jj in range(GRP):
                    for h in range(2):
                        nc.tensor.matmul(
                            pp[0:1, h * 512:(h + 1) * 512],
                            lhsT=c_g[:, jj:jj + 1],
                            rhs=xt[:, jj, h * 512:(h + 1) * 512],
                            start=(g == 0 and jj == 0),
                            stop=(g == NG - 1 and jj == GRP - 1),
                        )

            # denom = sum of e over all s
            dps = psum_small.tile([1, NT], f32, tag="sm")
            nc.tensor.matmul(dps, lhsT=ones, rhs=e_all, start=True, stop=True)
            dsum = small.tile([1, 1], f32, tag="dsum")
            nc.vector.reduce_sum(dsum, dps, axis=AX.X)
            invd = small.tile([1, 1], f32, tag="invd")
            nc.vector.reciprocal(invd, dsum)
            # pooled = pooled_raw * invd * ln_w
            nc.vector.scalar_tensor_tensor(
                out=pooled_sb[0:1, b, :], in0=pp[0:1, :], scalar=invd,
                in1=lnw_sb, op0=OP.mult, op1=OP.mult,
            )

        for b in range(NB):
            nc.sync.dma_start(out=pooled_d[b:b + 1, :],
                              in_=pooled_sb[0:1, b, :])

    nc.compile()
    return nc




def _build_nc_pool_t(npe8=0, nnat=0):
    """Transposed-layout pool NEFF. Host sends xT [NB, D, S] bf16.

    npe8 > 0: additionally ships the last npe8 128-s-blocks of each half
    in NATURAL [s, d] layout as fp8e4m3; their pooled contribution runs on
    the PE (flip matmuls with c columns as rhs), cutting the DVE TTR pass
    to (8 - npe8)/8 of the s-range at the cost of npe8/8 * 8MB extra DMA.

    Each batch is processed as two independent s-halves pipelined through
    DMA -> squares (ACT/Pool split) -> PE flip matmuls (q/t as [128,JS]
    psum columns) -> DVE softmax smalls -> PE transposes of c -> ACT row
    cast -> Pool partition broadcast -> DVE tensor_tensor_reduce (pooled),
    with a tiny per-batch tail (denominator + scale).  DVE is the critical
    engine (~1.13 ns/col for the pooled TTR pass); emission order keeps it
    busy across halves and batches.
    """
    import concourse.bacc as bacc
    import concourse.mybir as mybir
    import concourse.tile as tile
    from concourse import bass_isa
    from contextlib import ExitStack

    f32 = mybir.dt.float32
    bf16 = mybir.dt.bfloat16
    AF = mybir.ActivationFunctionType
    OP = mybir.AluOpType
    AX = mybir.AxisListType

    KD = D // P            # 8 d-chunks
    NH = 2                 # s-halves per batch
    SH = S // NH           # 1024 s per half
    JS = SH // P           # 8 s-blocks per half

    nc = bacc.Bacc("TRN2", target_bir_lowering=False, debug=False,
                   enable_asserts=False, num_devices=NCORES)

    xt_d = nc.dram_tensor("xt", [NB, D, S], bf16, kind="ExternalInput").ap()
    a_d = nc.dram_tensor("a", [P, KD], bf16, kind="ExternalInput").ap()
    lnw_d = nc.dram_tensor("lnw", [P, KD], f32, kind="ExternalInput").ap()
    eye_d = nc.dram_tensor("eye", [P, P], f32, kind="ExternalInput").ap()
    cst_d = nc.dram_tensor("cst", [1, 2], f32, kind="ExternalInput").ap()
    one16_d = nc.dram_tensor("one16", [1, 1], bf16, kind="ExternalInput").ap()
    f8 = mybir.dt.float8e4
    NPE = npe8
    NTT = JS - NPE          # s-blocks pooled via DVE TTR
    ST = NTT * P            # TTR s-range per half
    if NPE:
        x8_d = nc.dram_tensor("x8", [NB, NH, NPE * P, D], f8,
                              kind="ExternalInput").ap()
    pooled_d = nc.dram_tensor("pooled", [NB, P, KD], f32,
                              kind="ExternalOutput").ap()
    cscr_d = nc.dram_tensor("cscr", [NB * NH, 1, SH], bf16,
                            kind="Internal").ap()
    NTJ = JS - nnat       # j-columns pooled via DVE mul+reduce
    if nnat:
        xn_d = nc.dram_tensor("xn", [NB, NH, nnat, P, D], bf16,
                              kind="ExternalInput").ap()
        pn_d = nc.dram_tensor("pn", [NB, NH, 1, D], f32,
                              kind="ExternalOutput").ap()
        den_d = nc.dram_tensor("den", [NB, 1], f32,
                               kind="ExternalOutput").ap()

    with tile.TileContext(nc) as tc, ExitStack() as ctx:
        singles = ctx.enter_context(tc.tile_pool(name="singles", bufs=1))
        xpool = ctx.enter_context(tc.tile_pool(name="xp", bufs=4))
        sqpool = ctx.enter_context(tc.tile_pool(name="sqp", bufs=4))
        scrpool = ctx.enter_context(tc.tile_pool(name="scrp", bufs=2))
        cpool = ctx.enter_context(tc.tile_pool(name="cp", bufs=3))
        small = ctx.enter_context(tc.tile_pool(name="small", bufs=4))
        ps_qt = ctx.enter_context(tc.tile_pool(name="psqt", bufs=3, space="PSUM"))
        ps_ct = ctx.enter_context(
            tc.tile_pool(name="psct", bufs=1 if npe8 else 2, space="PSUM"))
        if npe8:
            x8pool = ctx.enter_context(tc.tile_pool(name="x8p", bufs=4))
            ps_nat = ctx.enter_context(
                tc.tile_pool(name="psnat", bufs=3, space="PSUM"))
        if nnat:
            xnpool = ctx.enter_context(tc.tile_pool(name="xnp", bufs=2))
            ps_pn = ctx.enter_context(
                tc.tile_pool(name="pspn", bufs=2, space="PSUM"))

        a_sb = singles.tile([P, KD], bf16)
        nc.sync.dma_start(out=a_sb, in_=a_d)
        lnw_sb = singles.tile([P, KD], f32)
        nc.sync.dma_start(out=lnw_sb, in_=lnw_d)
        eye_sb = singles.tile([P, P], f32)
        nc.sync.dma_start(out=eye_sb, in_=eye_d)
        ones16 = singles.tile([P, 1], bf16)
        nc.sync.dma_start(out=ones16, in_=one16_d.to_broadcast([P, 1]))
        ones32 = singles.tile([P, 1], f32)
        nc.sync.dma_start(out=ones32, in_=cst_d[0:1, 0:1].to_broadcast([P, 1]))

        # software pipeline over units u = (b, h):
        #   A(u): DMA + squares + q/t flip matmuls
        #   B(u): DVE softmax smalls -> sc, e32; Pool c32
        #   C(u): PE transposes of c32 -> ACT row cast -> Pool broadcast
        #   D(u): DVE pooled TTRs (half 1 seeded by half 0) + er reduce
        # Emission staggers stages so each unit's C-chain overlaps the
        # previous unit's TTR block on DVE.
        NU = NB * NH
        st = [dict() for _ in range(NU)]
        if nnat:
            pn_all = singles.tile([1, NB * NH, D], f32)
            den_all = singles.tile([1, NB], f32)

        def stage_a1(u):
            s = st[u]
            h = u % NH
            b = u // NH
            xt = xpool.tile([P, KD, SH], bf16, tag="x")
            if u < 2:
                # fill phase: split the 2MB load so squares start sooner
                nc.sync.dma_start(
                    out=xt[:, 0:KD // 2, :],
                    in_=xt_d[b, 0:D // 2, h * SH:(h + 1) * SH].rearrange(
                        "(k p) s -> p k s", p=P))
                nc.sync.dma_start(
                    out=xt[:, KD // 2:KD, :],
                    in_=xt_d[b, D // 2:D, h * SH:(h + 1) * SH].rearrange(
                        "(k p) s -> p k s", p=P))
            else:
                for g2 in range(2):
                    nc.sync.dma_start(
                        out=xt[:, g2 * (KD // 2):(g2 + 1) * (KD // 2), :],
                        in_=xt_d[b, g2 * (D // 2):(g2 + 1) * (D // 2),
                                 h * SH:(h + 1) * SH].rearrange(
                            "(k p) s -> p k s", p=P))
            if NPE:
                x8t = x8pool.tile([P, NPE, D], f8, tag="x8")
                nc.sync.dma_start(
                    out=x8t,
                    in_=x8_d[b, h].rearrange("(j p) d -> p j d", p=P))
                s["x8"] = x8t
            if nnat:
                if h == 1:
                    xn = xnpool.tile([P, NH * nnat, D], bf16, tag="xn")
                    nc.sync.dma_start(
                        out=xn,
                        in_=xn_d[b].rearrange("h j p d -> p (h j) d"))
                    s["xn"] = xn
                    st[u - 1]["xn"] = xn
                else:
                    s["xn"] = None  # filled by the h==1 unit
            sq = sqpool.tile([P, KD, SH], bf16, tag="sq")
            s["xt"] = xt
            s["sq"] = sq
            for k in range(KD):
                # steady state: ACT does 7, Pool 1 (Pool's square is
                # emitted in stage_a2, after the pooled muls). During
                # fill (first two units) DVE helps too.
                if u < 2:
                    eng = ("act", "act", "act", "dve", "dve", "pool",
                           "pool", "dve")[k]
                else:
                    eng = "pool" if (k == 5 or (nnat and k == 2)) else "act"
                if eng == "pool":
                    continue
                if eng == "dve":
                    nc.vector.tensor_mul(sq[:, k, :], xt[:, k, :],
                                         xt[:, k, :])
                else:
                    nc.scalar.activation(out=sq[:, k, :], in_=xt[:, k, :],
                                         func=AF.Square)

        def stage_a2(u):
            s = st[u]
            xt = s["xt"]
            sq = s["sq"]
            pool_ks = (5, 6) if u < 2 else ((5, 2) if nnat else (5,))
            for k in pool_ks:
                nc.gpsimd.tensor_mul(sq[:, k, :], xt[:, k, :],
                                     xt[:, k, :])
            # strided column slices: out row ps of block j <-> s-pos
            # m = ps*JS + j, matching c's flat DRAM order later
            xv = xt.rearrange("p k (ps j) -> p k j ps", j=JS)
            sv = sq.rearrange("p k (ps j) -> p k j ps", j=JS)
            qt_ps = ps_qt.tile([P, 2 * JS], f32, tag="qtps")
            for j in range(JS):
                for k in range(KD):
                    nc.tensor.matmul(
                        qt_ps[:, j:j + 1],
                        lhsT=xv[:, k, j, :],
                        rhs=a_sb[:, k:k + 1],
                        start=(k == 0), stop=(k == KD - 1))
                for k in range(KD):
                    nc.tensor.matmul(
                        qt_ps[:, JS + j:JS + j + 1],
                        lhsT=sv[:, k, j, :],
                        rhs=ones16,
                        start=(k == 0), stop=(k == KD - 1))
            s["qt"] = qt_ps

        def stage_b(u):
            s = st[u]
            qt_ps = s["qt"]
            t_ps = qt_ps[:, 0:JS]
            q_ps = qt_ps[:, JS:2 * JS]
            v = small.tile([P, JS], f32, tag="v")
            nc.vector.tensor_scalar(
                out=v, in0=q_ps, scalar1=1.0 / D, scalar2=EPS,
                op0=OP.mult, op1=OP.add)
            y = small.tile([P, JS], f32, tag="y")
            nc.vector.tensor_scalar(
                out=y, in0=v, scalar1=-0.5, scalar2=1.5,
                op0=OP.mult, op1=OP.add)
            u2 = small.tile([P, JS], f32, tag="u")
            nc.vector.tensor_mul(u2, y, y)
            nc.vector.tensor_mul(u2, u2, v)
            nc.vector.tensor_scalar(
                out=u2, in0=u2, scalar1=-0.5, scalar2=1.5,
                op0=OP.mult, op1=OP.add)
            nc.vector.tensor_mul(y, y, u2)
            sc = small.tile([P, JS], f32, tag="sc")
            nc.vector.tensor_mul(sc, t_ps, y)
            e32 = small.tile([P, JS], f32, tag="e")
            nc.scalar.activation(out=e32, in_=sc, func=AF.Exp)
            c16 = small.tile([P, JS], bf16, tag="c16")
            nc.vector.tensor_mul(c16, e32, y)
            nc.sync.dma_start(
                out=cscr_d[u].rearrange("o (p j) -> p (o j)", p=P),
                in_=c16)
            c_row = cpool.tile([1, SH], bf16, tag="crow")
            nc.sync.dma_start(out=c_row, in_=cscr_d[u])
            s["crow"] = c_row
            s["c16"] = c16
            s["e32"] = e32

        def stage_c(u):
            s = st[u]
            c_bc = cpool.tile([P, SH], bf16, tag="cbc")
            nc.gpsimd.partition_broadcast(c_bc, s["crow"])
            s["cbc"] = c_bc

        def stage_d(u):
            s = st[u]
            h = u % NH
            er = small.tile([P, 1], f32, tag="er")
            nc.vector.reduce_sum(er, s["e32"], axis=AX.X)
            s["er"] = er
            praw = small.tile([P, KD], f32,
                              tag="praw0" if h == 0 else "praw")
            s["praw"] = praw
            if not nnat:
                for k in range(KD):
                    prod = scrpool.tile([P, SH], bf16, tag="prod")
                    nc.gpsimd.tensor_mul(prod, s["xt"][:, k, :], s["cbc"])
                    nc.vector.reduce_sum(praw[:, k:k + 1], prod, axis=AX.X)
            else:
                cbv = s["cbc"].rearrange("p (ps j) -> p ps j", j=JS)
                for k in range(KD):
                    xkv = s["xt"][:, k].rearrange("p (ps j) -> p ps j", j=JS)
                    prod = scrpool.tile([P, P * NTJ], bf16, tag="prod")
                    pv = prod.rearrange("p (ps j) -> p ps j", j=NTJ)
                    nc.gpsimd.tensor_mul(pv, xkv[:, :, 0:NTJ],
                                         cbv[:, :, 0:NTJ])
                    nc.vector.reduce_sum(praw[:, k:k + 1], prod, axis=AX.X)
                # natural-layout pooled for the last nnat j-columns:
                # classic orientation, c16 column as lhsT (d-linear row out,
                # shipped raw; host applies lnw/denom)
                c16 = s["c16"]
                pn_ps = ps_pn.tile([1, D], f32, tag="pn")
                joff = h * nnat
                for h2 in range(2):
                    for jj in range(nnat):
                        nc.tensor.matmul(
                            pn_ps[0:1, h2 * 512:(h2 + 1) * 512],
                            lhsT=c16[:, NTJ + jj:NTJ + jj + 1],
                            rhs=s["xn"][:, joff + jj,
                                        h2 * 512:(h2 + 1) * 512],
                            start=(jj == 0), stop=(jj == nnat - 1))
                # copy into the batched output row; one DMA at the end
                nc.scalar.activation(out=pn_all[0:1, u, :], in_=pn_ps,
                                     func=AF.Copy)

        def tail(b):
            s0, s1 = st[2 * b], st[2 * b + 1]
            er_tot = small.tile([P, 1], f32, tag="ertot")
            nc.vector.tensor_add(er_tot, s0["er"], s1["er"])
            dall = small.tile([P, 1], f32, tag="dall")
            nc.gpsimd.partition_all_reduce(
                dall, er_tot, channels=P, reduce_op=bass_isa.ReduceOp.add)
            invd_bc = small.tile([P, 1], f32, tag="invdbc")
            nc.vector.reciprocal(invd_bc, dall)
            if nnat:
                nc.vector.tensor_copy(den_all[0:1, b:b + 1],
                                      dall[0:1, 0:1])
            praw_f = s1["praw"]
            nc.vector.tensor_add(praw_f, praw_f, s0["praw"])
            pooled_sb = small.tile([P, KD], f32, tag="pooled")
            nc.vector.scalar_tensor_tensor(
                out=pooled_sb, in0=praw_f, scalar=invd_bc, in1=lnw_sb,
                op0=OP.mult, op1=OP.mult)
            nc.sync.dma_start(out=pooled_d[b], in_=pooled_sb)

        def flush_outputs():
            if nnat:
                nc.sync.dma_start(
                    out=pn_d.rearrange("b h o d -> o (b h) d"),
                    in_=pn_all)
                nc.sync.dma_start(
                    out=den_d.rearrange("b o -> o b"), in_=den_all)

        for i in range(NU + 2):
            if i < NU:
                stage_a1(i)
            if 0 <= i - 1 < NU:
                stage_b(i - 1)
            if 0 <= i - 2 < NU:
                stage_d(i - 2)
            if i < NU:
                stage_a2(i)
            if 0 <= i - 1 < NU:
                stage_c(i - 1)
            if 0 <= i - 2 < NU:
                if (i - 2) % NH == NH - 1:
                    tail((i - 2) // NH)
        flush_outputs()

    nc.compile()
    return nc


def _pool_t_in_maps(x, ln_w, att_w, npe8=0, nnat=0):
    import ml_dtypes
    KD = D // P
    NH = 2
    SH = S // NH
    ST = SH - npe8 * P
    a_lin = (ln_w * att_w[:, 0]).astype(np.float32)
    a_col = np.ascontiguousarray(
        a_lin.reshape(KD, P).T).astype(ml_dtypes.bfloat16)      # [P, KD]
    lnw_col = np.ascontiguousarray(
        ln_w.astype(np.float32).reshape(KD, P).T)               # [P, KD]
    eye = np.eye(P, dtype=np.float32)
    cst = np.array([[1.0, EPS]], dtype=np.float32)
    one16 = np.ones((1, 1), dtype=np.float32).astype(ml_dtypes.bfloat16)
    xt_all = np.ascontiguousarray(
        x.transpose(0, 2, 1).astype(ml_dtypes.bfloat16))        # [B, D, S]
    maps = [
        {"xt": xt_all[r * NB:(r + 1) * NB], "a": a_col, "lnw": lnw_col,
         "eye": eye, "cst": cst, "one16": one16}
        for r in range(NCORES)
    ]
    if npe8:
        x8_all = np.ascontiguousarray(np.stack(
            [x[:, h * SH + ST:(h + 1) * SH, :] for h in range(NH)],
            axis=1).astype(ml_dtypes.float8_e4m3))   # [B, NH, npe8*P, D]
        for r in range(NCORES):
            maps[r]["x8"] = x8_all[r * NB:(r + 1) * NB]
    if nnat:
        JS = SH // P
        NTJ = JS - nnat
        # xn[b,h,jj,p,:] = x[b, h*SH + p*JS + (NTJ+jj), :]
        xn_all = np.ascontiguousarray(np.stack(
            [np.stack([x[:, h * SH + NTJ + jj::JS, :][:, :P, :]
                       for jj in range(nnat)], axis=1)
             for h in range(NH)], axis=1).astype(ml_dtypes.bfloat16))
        for r in range(NCORES):
            maps[r]["xn"] = xn_all[r * NB:(r + 1) * NB]
    return maps


def _pool_t_unshard(res, ln_w=None, nnat=0):
    """pooled dram [NB, P, KD] per core -> full [B, D] with d = k*P + p.
    nnat: add the raw natural-layout rows, scaled by lnw/denom host-side."""
    KD = D // P
    parts = []
    for r in range(NCORES):
        pr = res.results[r]["pooled"]              # [NB, P, KD]
        part = np.ascontiguousarray(
            pr.transpose(0, 2, 1)).reshape(NB, D).astype(np.float64)
        if nnat:
            pn = res.results[r]["pn"].astype(np.float64)   # [NB, NH, 1, D]
            den = res.results[r]["den"].astype(np.float64)  # [NB, 1]
            part = part + pn.sum(axis=1)[:, 0, :] * ln_w[None, :] / den
        parts.append(part.astype(np.float32))
    return np.concatenate(parts, axis=0)


def _build_nc_pool_classic():
    """Conservative pool NEFF: fp32 x, per-tile DMAs, per-batch softmax,
    fp32 matmuls — mirrors the structure already proven to execute on HW."""
    import concourse.bacc as bacc
    import concourse.mybir as mybir
    import concourse.tile as tile
    from contextlib import ExitStack

    f32 = mybir.dt.float32
    AF = mybir.ActivationFunctionType
    OP = mybir.AluOpType
    AX = mybir.AxisListType

    nc = bacc.Bacc("TRN2", target_bir_lowering=False, debug=False,
                   enable_asserts=False, num_devices=NCORES)

    x_d = nc.dram_tensor("x", [NB, S, D], f32, kind="ExternalInput").ap()
    a_d = nc.dram_tensor("a", [1, D], f32, kind="ExternalInput").ap()
    lnw_d = nc.dram_tensor("lnw", [1, D], f32, kind="ExternalInput").ap()
    cst_d = nc.dram_tensor("cst", [1, 2], f32, kind="ExternalInput").ap()
    pooled_d = nc.dram_tensor("pooled", [NB, D], f32, kind="ExternalOutput").ap()

    with tile.TileContext(nc) as tc, ExitStack() as ctx:
        singles = ctx.enter_context(tc.tile_pool(name="singles", bufs=1))
        xpool = ctx.enter_context(tc.tile_pool(name="xp", bufs=26))
        scratch = ctx.enter_context(tc.tile_pool(name="scr", bufs=2))
        small = ctx.enter_context(tc.tile_pool(name="small", bufs=3))
        psum_pool = ctx.enter_context(tc.tile_pool(name="pspool", bufs=2, space="PSUM"))
        psum_small = ctx.enter_context(tc.tile_pool(name="pssm", bufs=2, space="PSUM"))

        a_bc = singles.tile([P, D], f32)
        nc.sync.dma_start(out=a_bc, in_=a_d.to_broadcast([P, D]))
        lnw_sb = singles.tile([1, D], f32)
        nc.sync.dma_start(out=lnw_sb, in_=lnw_d)
        # constants via DMA broadcast (DVE memset is unreliable on this runtime)
        ones = singles.tile([P, 1], f32)
        nc.sync.dma_start(out=ones, in_=cst_d[0:1, 0:1].to_broadcast([P, 1]))
        eps_col = singles.tile([P, 1], f32)
        nc.sync.dma_start(out=eps_col, in_=cst_d[0:1, 1:2].to_broadcast([P, 1]))

        pooled_sb = singles.tile([1, NB, D], f32)

        for b in range(NB):
            q_all = small.tile([P, NT], f32, tag="q")
            t_all = small.tile([P, NT], f32, tag="t")
            x_tiles = []
            for j in range(NT):
                xt = xpool.tile([P, D], f32, tag="x")
                nc.sync.dma_start(out=xt, in_=x_d[b, j * P:(j + 1) * P, :])
                x_tiles.append(xt)
                sq = scratch.tile([P, D], f32, tag="sq")
                nc.scalar.activation(out=sq, in_=xt, func=AF.Square)
                nc.vector.reduce_sum(q_all[:, j:j + 1], sq, axis=AX.X)
                tp = scratch.tile([P, D], f32, tag="tp")
                nc.vector.tensor_mul(tp, xt, a_bc)
                nc.vector.reduce_sum(t_all[:, j:j + 1], tp, axis=AX.X)

            # rs = 1/sqrt(q/D + eps)  (groupnorm's sqrt+reciprocal recipe)
            rs = small.tile([P, NT], f32, tag="rs")
            nc.scalar.activation(out=rs, in_=q_all, func=AF.Sqrt,
                                 scale=1.0 / D, bias=eps_col)
            nc.vector.reciprocal(rs, rs)
            sc = small.tile([P, NT], f32, tag="sc")
            nc.vector.tensor_mul(sc, t_all, rs)
            e_all = small.tile([P, NT], f32, tag="e")
            nc.scalar.activation(out=e_all, in_=sc, func=AF.Exp)
            c_all = small.tile([P, NT], f32, tag="c")
            nc.vector.tensor_mul(c_all, e_all, rs)

            dps = psum_small.tile([1, NT], f32, tag="sm")
            nc.tensor.matmul(dps, lhsT=ones, rhs=e_all, start=True, stop=True)
            dsum = small.tile([1, 1], f32, tag="dsum")
            nc.vector.reduce_sum(dsum, dps, axis=AX.X)
            invd = small.tile([1, 1], f32, tag="invd")
            nc.vector.reciprocal(invd, dsum)

            pp = psum_pool.tile([1, D], f32, tag="acc")
            for j in range(NT):
                for h in range(2):
                    nc.tensor.matmul(
                        pp[0:1, h * 512:(h + 1) * 512],
                        lhsT=c_all[:, j:j + 1],
                        rhs=x_tiles[j][:, h * 512:(h + 1) * 512],
                        start=(j == 0), stop=(j == NT - 1))
            nc.vector.scalar_tensor_tensor(
                out=pooled_sb[0:1, b, :], in0=pp[0:1, :], scalar=invd,
                in1=lnw_sb, op0=OP.mult, op1=OP.mult)

        for b in range(NB):
            nc.sync.dma_start(out=pooled_d[b:b + 1, :],
                              in_=pooled_sb[0:1, b, :])

    nc.compile()
    return nc

def _build_nc_geglu(mm="bf16x2"):
    import concourse.bacc as bacc
    import concourse.mybir as mybir
    import concourse.tile as tile
    from contextlib import ExitStack

    f32 = mybir.dt.float32
    bf16 = mybir.dt.bfloat16
    comp = mm == "bf16x2"   # compensated bf16: hi/lo split of both operands
    mdt = f32 if mm == "fp32" else bf16
    NIN = 2 if comp else 1
    AF = mybir.ActivationFunctionType

    nc = bacc.Bacc(
        "TRN2",
        target_bir_lowering=False,
        debug=False,
        enable_asserts=False,
        num_devices=NCORES,
    )

    pT_d = nc.dram_tensor("pT", [P, NIN, 8, B], mdt, kind="ExternalInput").ap()
    w_d = nc.dram_tensor("w", [NIN, 8, P, 2 * COLS], mdt, kind="ExternalInput").ap()
    bias_d = nc.dram_tensor("bias", [1, 2 * COLS], f32, kind="ExternalInput").ap()
    out_d = nc.dram_tensor("out", [B, COLS], f32, kind="ExternalOutput").ap()

    with tile.TileContext(nc) as tc, ExitStack() as ctx:
        singles = ctx.enter_context(tc.tile_pool(name="singles", bufs=1))
        tailp = ctx.enter_context(tc.tile_pool(name="tail", bufs=2))
        psum_pool = ctx.enter_context(
            tc.tile_pool(name="pspool", bufs=1, space="PSUM")
        )

        pT_sb = singles.tile([P, NIN, 8, B], mdt)
        nc.sync.dma_start(out=pT_sb, in_=pT_d)
        # per-chunk DMAs so matmul k can start as soon as chunk k lands
        w_sb = singles.tile([P, NIN, 8, 2 * COLS], mdt)
        for n in range(NIN):
            for k in range(8):
                nc.sync.dma_start(out=w_sb[:, n, k], in_=w_d[n, k])
        bias_bc = singles.tile([B, 2 * COLS], f32)
        nc.sync.dma_start(out=bias_bc, in_=bias_d.to_broadcast([B, 2 * COLS]))

        # terms: hi@hi (+ lo@hi + hi@lo when compensated); the w_lo term
        # goes last since the lo half of W streams in after the hi half
        terms = [(0, 0)] if not comp else [(0, 0), (1, 0), (0, 1)]
        hps = psum_pool.tile([B, 2 * COLS], f32, tag="acc")
        for ti, (pn, wn) in enumerate(terms):
            for k in range(8):
                for h in range(2):
                    nc.tensor.matmul(
                        hps[:, h * COLS:(h + 1) * COLS],
                        lhsT=pT_sb[:, pn, k, :],
                        rhs=w_sb[:, wn, k, h * COLS:(h + 1) * COLS],
                        start=(ti == 0 and k == 0),
                        stop=(ti == len(terms) - 1 and k == 7),
                    )
        hv = tailp.tile([B, COLS], f32, tag="hv")
        nc.vector.tensor_add(hv, hps[:, 0:COLS], bias_bc[:, 0:COLS])
        hg = tailp.tile([B, COLS], f32, tag="hg")
        nc.vector.tensor_add(hg, hps[:, COLS:2 * COLS], bias_bc[:, COLS:2 * COLS])
        gg = tailp.tile([B, COLS], f32, tag="gg")
        nc.scalar.activation(out=gg, in_=hg,
                             func=AF.Tanh if SIM_SAFE_GELU else AF.Gelu)
        outt = tailp.tile([B, COLS], f32, tag="outt")
        nc.vector.tensor_mul(outt, hv, gg)
        nc.sync.dma_start(out=out_d, in_=outt)

    nc.compile()
    return nc


def _pool_in_maps(x, ln_w, att_w, mm="xbf16"):
    import ml_dtypes
    xdt = ml_dtypes.bfloat16 if mm == "xbf16" else np.float32
    if mm == "classic":
        xdt = np.float32
    a = (ln_w * att_w[:, 0]).astype(xdt).reshape(1, D)
    lnw = ln_w.astype(np.float32).reshape(1, D)
    xc = np.ascontiguousarray(x.astype(xdt))
    cst = np.array([[1.0, EPS]], dtype=np.float32)
    return [
        {"x": xc[r * NB:(r + 1) * NB], "a": a, "lnw": lnw, "cst": cst}
        for r in range(NCORES)
    ]


def _split_hi_lo(arr, comp):
    import ml_dtypes
    if not comp:
        return arr.astype(ml_dtypes.bfloat16)[None]
    hi = arr.astype(ml_dtypes.bfloat16)
    lo = (arr - hi.astype(np.float32)).astype(ml_dtypes.bfloat16)
    return np.stack([hi, lo])


def _geglu_in_maps(pooled_full, geglu_w, geglu_b, mm="bf16x2"):
    comp = mm == "bf16x2"
    NIN = 2 if comp else 1
    if mm == "fp32":
        def conv(a):
            return a.astype(np.float32)[None]
    else:
        def conv(a):
            return _split_hi_lo(a, comp)
    pTn = np.ascontiguousarray(
        conv(np.ascontiguousarray(pooled_full.T))
    ).reshape(NIN, 8, P, B)
    pT = np.ascontiguousarray(np.transpose(pTn, (2, 0, 1, 3)))
    maps = []
    for r in range(NCORES):
        vs = slice(r * COLS, (r + 1) * COLS)
        gs = slice(OUT + r * COLS, OUT + (r + 1) * COLS)
        wcat = np.ascontiguousarray(
            np.concatenate([geglu_w[:, vs], geglu_w[:, gs]], axis=1)
        )
        wr = np.ascontiguousarray(conv(wcat)).reshape(NIN, 8, P, 2 * COLS)
        br = np.ascontiguousarray(
            np.concatenate([geglu_b[vs], geglu_b[gs]])
        ).reshape(1, 2 * COLS)
        maps.append({"pT": pT, "w": wr, "bias": br})
    return maps


LAST_RESULTS = None


def kernel(x, ln_w, att_w, att_b, geglu_w, geglu_b):
    global LAST_RESULTS
    from concourse.bass_utils import run_bass_kernel_spmd

    x = np.asarray(x, dtype=np.float32)
    ln_w = np.asarray(ln_w, dtype=np.float32)
    att_w = np.asarray(att_w, dtype=np.float32)
    geglu_w = np.asarray(geglu_w, dtype=np.float32)
    geglu_b = np.asarray(geglu_b, dtype=np.float32)
    # att_b is mathematically irrelevant (softmax shift-invariance)

    mm = os.environ.get("KERNEL_MM", "tposen")
    gg = os.environ.get("KERNEL_GG", "bf16")
    npe8 = int(os.environ.get("KERNEL_NPE8", "3")) if mm == "tpose8" else 0
    nnat = int(os.environ.get("KERNEL_NNAT", "2")) if mm == "tposen" else 0
    if ("A", mm) not in _cache:
        if mm == "classic":
            _cache[("A", mm)] = _build_nc_pool_classic()
        elif mm == "tpose":
            _cache[("A", mm)] = _build_nc_pool_t()
        elif mm == "tpose8":
            _cache[("A", mm)] = _build_nc_pool_t(npe8=npe8)
        elif mm == "tposen":
            _cache[("A", mm)] = _build_nc_pool_t(nnat=nnat)
        else:
            _cache[("A", mm)] = _build_nc_pool(mm=mm)
    if ("B", gg) not in _cache:
        _cache[("B", gg)] = _build_nc_geglu(mm=gg)

    trace = os.environ.get("KERNEL_TRACE", "0") == "1"

    if mm in ("tpose", "tpose8", "tposen"):
        in_maps_a = _pool_t_in_maps(x, ln_w, att_w, npe8=npe8, nnat=nnat)
    else:
        in_maps_a = _pool_in_maps(x, ln_w, att_w, mm=mm)
    res_a = run_bass_kernel_spmd(
        _cache[("A", mm)], in_maps_a,
        core_ids=list(range(NCORES)), trace=trace,
    )
    if mm in ("tpose", "tpose8", "tposen"):
        pooled_full = _pool_t_unshard(res_a, ln_w=ln_w, nnat=nnat)
    else:
        pooled_full = np.concatenate(
            [res_a.results[r]["pooled"] for r in range(NCORES)], axis=0
        )
    res_b = run_bass_kernel_spmd(
        _cache[("B", gg)], _geglu_in_maps(pooled_full, geglu_w, geglu_b, mm=gg),
        core_ids=list(range(NCORES)), trace=trace,
    )
    LAST_RESULTS = (res_a, res_b)
    out = np.concatenate(
        [res_b.results[r]["out"] for r in range(NCORES)], axis=1
    )
    return out.astype(np.float32)

